# revision 44
# baseline (speedup 1.0000x reference)
"""Causal attention (B=4, S=2048, D=1024, fp32) on 8 Trainium2 NeuronCores.

Sharding: data-parallel over batch (4) x query-split (2) per batch. The two
cores of a batch take interleaved query rows (even/odd within each 512-row
super-block), which makes the causal workload identical on every core and
lets one SPMD program serve all 8 cores; the only per-core differences are
pure data (which query columns of x^T each core receives, and the mask
tiles, which carry the even/odd offset).

Two weight-only fusions remove most projection work from the device:
 1. scores = (x Wq)(x Wk)^T = x (Wq Wk^T) x^T: the host precomputes
    M = Wq Wk^T, so x^T itself is the key matrix (no K projection) and the
    only query-side work is q' = M^T x_own.
 2. ctx = softmax(scores) (x Wv) = (softmax(scores) x) Wv: the context is
    accumulated in x-space, transposed (U^T[d,q] = sum_k x[k,d] p[k,q]), and
    Wv is applied once per 128-query group at slot end — 1024 queries
    instead of 2048 keys pay for the V projection, and the out-projection
    result lands in [q, e] layout where the per-query softmax normalize is
    a native per-partition scale.

Schedule (the perf-critical part):
 - The critical-path inputs (m, xq, kx) ride the two fast HWDGE queues
   (sync + scalar) in consumption order; bulk late-deadline inputs (wv,
   xkd groups 1-3) go via the slower GpSimd SWDGE queue. Scalar's dispatch
   work finishes ~15us in, long before its first PSUM->SBUF copies.
 - q' projection runs dc-outer with 8 concurrent PSUM accumulation groups,
   so the PE starts on the first 256KB chunk pair and is paced by DMA
   arrival, never blocked on the full 4MB.
 - qT is split into per-jq tiles (qTa for slots 0-1, qTb for slots 2-3) so
   slot-0 scores depend only on the first projection wave's copies (tile
   deps are whole-tile, not region).
 - One flat score->exp->ctx pipeline runs across slot boundaries (the
   lookahead queue spans slots), so next-slot scores are in flight while
   the previous slot's U^T drains through copies + out-projection.
 - Diagonal score/ctx tiles are column-trimmed to the causally live range
   [64*t, 256): stale PSUM in the trimmed columns is finite, the full-width
   mask add writes -1e30 there, and exp maps them to exactly 0.

All matmul inputs are bf16 (cast host-side), accumulation in fp32 PSUM.
Measured rel err ~5e-3 vs the fp32 reference (gate 2e-2).
"""

import numpy as np

B, S, D = 4, 2048, 1024
NE = D // 128          # contraction chunks (d on partitions)
NKBLK = S // 128       # 128-wide key blocks
NSLOT = 4              # query slots per core
QW = 256               # queries per slot
OWNQ = NSLOT * QW      # 1024 queries per core
MASK_NEG = -1.0e30
SCALE = 1.0 / 32.0     # 1/sqrt(D)

_cached = {}


def _build():
    import concourse.bacc as bacc
    import concourse.tile as tile
    import concourse.mybir as mybir
    from concourse.tile_rust import add_dep_helper
    from collections import deque

    F32 = mybir.dt.float32
    BF16 = mybir.dt.bfloat16
    EXP = mybir.ActivationFunctionType.Exp

    nc = bacc.Bacc("TRN2", target_bir_lowering=False, debug=False, num_devices=8,
                   dynamic_dma_scratch_size=2048)

    # Host-relayout inputs: each dram tensor region maps 1:1 onto one SBUF
    # tile so every dma_start is a single full-tile transfer. xkd group 0
    # is split in two so slot-0's first ctx blocks aren't gated on the
    # whole 1MB.
    mqx_d = nc.dram_tensor("mqx", [NE * 128, 2048], BF16, kind="ExternalInput")
    kxa_d = nc.dram_tensor("kxa", [128, 4 * S], BF16, kind="ExternalInput")
    kxb_d = nc.dram_tensor("kxb", [128, 4 * S], BF16, kind="ExternalInput")
    # xkd tile groups: kb blocks [0,2), [2,4), [4,8), [8,12), [12,16)
    XKG = [(0, 2), (2, 4), (4, 8), (8, 12), (12, 16)]
    xkdg_d = [nc.dram_tensor(f"xkd{g}", [128, (hi - lo) * D], BF16,
                             kind="ExternalInput")
              for g, (lo, hi) in enumerate(XKG)]
    wv_d = nc.dram_tensor("wv", [128, NE * D], BF16, kind="ExternalInput")
    mask_d = nc.dram_tensor("masks", [128, 4 * QW], F32, kind="ExternalInput")
    ones_d = nc.dram_tensor("ones", [128, 2], BF16, kind="ExternalInput")
    o_d = nc.dram_tensor("o", [OWNQ, D], F32, kind="ExternalOutput")

    with tile.TileContext(nc) as tc:
        with (
            tc.tile_pool(name="res", bufs=1) as res,
            tc.tile_pool(name="ptp", bufs=5) as ptp,
            tc.tile_pool(name="utp", bufs=2) as utp,
            tc.tile_pool(name="obp", bufs=2) as obp,
            tc.tile_pool(name="rcp", bufs=2) as rcp,
        ):
            # ---- resident tiles ----
            mqx = [res.tile([128, 2048], BF16, name=f"mqx{dc}", tag=f"mqx{dc}")
                   for dc in range(NE)]
            kxa = res.tile([128, 4 * S], BF16, name="kxa", tag="kxa")
            kxb = res.tile([128, 4 * S], BF16, name="kxb", tag="kxb")
            xkdg = [res.tile([128, (hi - lo) * D], BF16, name=f"xkdg{g}",
                             tag=f"xkdg{g}") for g, (lo, hi) in enumerate(XKG)]

            def xk_slice(kb, dc):
                for g, (lo, hi) in enumerate(XKG):
                    if lo <= kb < hi:
                        base = (kb - lo) * D + dc * 128
                        return xkdg[g][:, base:base + 128]
                raise AssertionError(kb)
            # q' split by jq wave: qTa serves slots 0-1, qTb slots 2-3
            qTa = [res.tile([128, 2 * QW], BF16, name=f"qTa{c}", tag=f"qTa{c}")
                   for c in range(NE)]
            qTb = [res.tile([128, 2 * QW], BF16, name=f"qTb{c}", tag=f"qTb{c}")
                   for c in range(NE)]
            wv_t = res.tile([128, NE * D], BF16, name="wv_t", tag="wv_t")
            mask_t = res.tile([128, 4 * QW], F32, name="mask_t", tag="mask_t")
            ones_t = res.tile([128, 2], BF16, name="ones_t", tag="ones_t")

            # ---- input DMAs in consumption order ----
            # DMA engines fair-share bandwidth across ALL in-flight
            # transfers, and each queue keeps ~4 in flight, so anything
            # dispatched early competes with the critical m|xq chunks.
            # The two HWDGE queues carry the critical prefix in deadline
            # order (the 4-slot window then self-clocks admissions); the
            # SWDGE (gpsimd) transfers are emitted later, anchored behind
            # compute instructions, so they launch only once the prefix
            # crunch has passed (see _delayed_dmas below).
            for dc in range(NE):
                e = nc.sync if dc % 2 == 0 else nc.scalar
                e.dma_start(mqx[dc][:, :], mqx_d[dc * 128:(dc + 1) * 128, :])
            nc.sync.dma_start(kxb[:, :], kxb_d[:, :])
            nc.scalar.dma_start(kxa[:, :], kxa_d[:, :])
            nc.scalar.dma_start(mask_t[:, :], mask_d[:, :])
            nc.sync.dma_start(xkdg[0][:, :], xkdg_d[0][:, :])
            nc.sync.dma_start(xkdg[1][:, :], xkdg_d[1][:, :])
            nc.gpsimd.dma_start(ones_t[:, :], ones_d[:, :])

            def delayed_dma(dst, src, anchor, why):
                d = nc.gpsimd.dma_start(dst, src)
                add_dep_helper(d.ins, anchor.ins, sync=True, reason=why)

            # ---- PSUM pools, shared by the q' projection and the slots so
            # every cross-phase dependency is tag-granular (a separate
            # q-proj pool scope made slot-0 wait on the pool release = ALL
            # 16 copies; this way each tile generation only waits its own
            # predecessor's reader).
            with (
                tc.tile_pool(name="rot", bufs=3, space="PSUM") as rot,
                tc.tile_pool(name="ctxp", bufs=1, space="PSUM") as ctxp,
                tc.tile_pool(name="dnp", bufs=1, space="PSUM") as dnp,
            ):
                # ---- q' projection, dc-outer: 7 concurrent PSUM groups
                # (ut0-3 + 3 rps rotations) consume each m|xq chunk as it
                # arrives; 16 (jq, ei) groups run in 3 waves.
                # qT{a,b}[ei] = sum_dc M_chunk^T xq_chunk
                groups = [(jq, ei) for jq in (0, 1) for ei in range(NE)]
                for wstart in range(0, 16, 8):
                    wave = groups[wstart:wstart + 8]
                    wtiles = []
                    for g in range(len(wave)):
                        if g < 4:
                            wtiles.append(ctxp.tile([128, 512], F32,
                                                    name=f"ut{g}", tag=f"ut{g}"))
                        elif g < 7:
                            wtiles.append(rot.tile([128, 512], F32,
                                                   name="rps", tag="rps"))
                        else:
                            # the dn bank doubles as the 8th group slot (the
                            # dn tag is sized to the max tile = one bank
                            # either way)
                            wtiles.append(dnp.tile([128, 512], F32,
                                                   name="dn", tag="dn"))
                    for dc in range(NE):
                        for g, (jq, ei) in enumerate(wave):
                            mm = nc.tensor.matmul(
                                wtiles[g][:, :],
                                mqx[dc][:, ei * 128:(ei + 1) * 128],
                                mqx[dc][:, 1024 + jq * 512:1024 + (jq + 1) * 512],
                                start=(dc == 0), stop=(dc == NE - 1),
                                skip_group_check=True,
                            )
                            if wstart == 0 and g == len(wave) - 1 and dc == 5:
                                # launch SWDGE bulk only once the HWDGE
                                # prefix crunch has passed
                                delayed_dma(wv_t[:, :], wv_d[:, :],
                                            mm, "wv after mqx")
                    for g, (jq, ei) in enumerate(wave):
                        qT = qTa if jq == 0 else qTb
                        # half-copies on DVE+ACT in parallel: the next
                        # wave's bank reuse (WAR on the whole tile) then
                        # waits ~350ns instead of a serial 680ns copy
                        for hf in range(2):
                            sl = slice(hf * 256, (hf + 1) * 256)
                            if (g + hf) % 2 == 0:
                                nc.vector.tensor_copy(qT[ei][:, sl],
                                                      wtiles[g][:, sl])
                            else:
                                nc.scalar.copy(qT[ei][:, sl],
                                               wtiles[g][:, sl])

                # ---- attention: one flat pipeline across all (slot,
                # key-block) pairs; the lookahead queue spans slot
                # boundaries so next-slot scores are emitted before the
                # previous slot's drain.
                state = {}  # slot -> (ut tiles, dn tile)

                def consume(item):
                    s, kb, off, pt = item
                    nk = 4 * s + 4
                    if kb == 0:
                        ut = [ctxp.tile([128, 512], F32, name=f"ut{t}",
                                        tag=f"ut{t}") for t in range(4)]
                        dn = dnp.tile([128, 4], F32, name="dn", tag="dn")
                        state[s] = (ut, dn)
                    ut, dn = state[s]
                    # U^T accumulation: ut[t] holds d-chunks 2t (cols 0:256)
                    # and 2t+1 (cols 256:512); the odd chunk lands on the bank
                    # the even chunk's start already cleared. Diagonal blocks
                    # only touch the causally live query range [off, QW).
                    for dc in range(NE):
                        t, half = dc // 2, dc % 2
                        nc.tensor.matmul(
                            ut[t][:, half * QW + off:(half + 1) * QW],
                            xk_slice(kb, dc),
                            pt[:, off:QW],
                            start=(kb == 0 and half == 0), stop=(kb == nk - 1),
                            skip_group_check=True,
                        )
                    for c in range(2):
                        # dn's two column groups share one PSUM bank; start=True
                        # clears the whole bank, so only the first group sets it.
                        nc.tensor.matmul(
                            dn[:, 2 * c:2 * c + 2],
                            pt[:, c * 128:(c + 1) * 128],
                            ones_t[:, :],
                            start=(kb == 0 and c == 0), stop=(kb == nk - 1),
                            skip_group_check=True,
                        )
                    if kb == nk - 1:
                        rc = rcp.tile([128, 2], F32, name="rc", tag="rc")
                        nc.vector.reciprocal(rc[:, :], dn[:, 0:4:2])
                        # U^T -> SBUF (bf16) in half-tile copies so the first
                        # out-projection weights load as early as possible,
                        # then o_raw[q,e] = sum_d U^T[d,q] Wv[d,e], normalize
                        # by 1/denom, DMA out.
                        # U^T -> SBUF in quarter-copies, c=0's quarters
                        # (cols 0:128 / 256:384 of each tile) first: the
                        # first out-projection matmul only needs one
                        # [128,128] copy (~200ns) instead of a half tile.
                        ut_sb = [utp.tile([128, 512], BF16, name=f"uts{t}",
                                          tag=f"uts{t}") for t in range(4)]
                        ci = 0
                        for qgrp in ((0, 2), (1, 3)):
                            for t in range(4):
                                for q in qgrp:
                                    sl = slice(q * 128, (q + 1) * 128)
                                    if ci % 2 == 0:
                                        nc.vector.tensor_copy(
                                            ut_sb[t][:, sl], ut[t][:, sl])
                                    else:
                                        nc.scalar.copy(
                                            ut_sb[t][:, sl], ut[t][:, sl])
                                    ci += 1
                        for c in range(2):
                            ob = obp.tile([128, D], F32, name="ob", tag="ob")
                            for dh in range(2):
                                ops = rot.tile([128, 512], F32, name="rps",
                                               tag="rps")
                                for dc in range(NE):
                                    nc.tensor.matmul(
                                        ops[:, :],
                                        ut_sb[dc // 2][:, (dc % 2) * QW + c * 128:
                                                       (dc % 2) * QW + (c + 1) * 128],
                                        wv_t[:, dc * D + dh * 512:
                                             dc * D + (dh + 1) * 512],
                                        start=(dc == 0), stop=(dc == NE - 1),
                                    )
                                # the very last chunk is normalized + DMAd in
                                # halves so the kernel-final DMA is 128KB.
                                # Slot 3's outs ride the scalar queue, whose
                                # dispatch sem slots are all free by then
                                # (sync still has input/out transfers in its
                                # 4-slot window).
                                nh = 2 if (s == NSLOT - 1 and c == 1 and
                                           dh == 1) else 1
                                oq = nc.scalar if s == NSLOT - 1 else nc.sync
                                for half in range(nh):
                                    w = 512 // nh
                                    lo = dh * 512 + half * w
                                    nc.vector.tensor_scalar_mul(
                                        ob[:, lo:lo + w],
                                        ops[:, half * w:(half + 1) * w],
                                        rc[:, c:c + 1],
                                    )
                                    oq.dma_start(
                                        o_d[s * QW + c * 128:
                                            s * QW + (c + 1) * 128,
                                            lo:lo + w],
                                        ob[:, lo:lo + w],
                                    )

                pending = deque()
                DEPTH = 4
                for s in range(NSLOT):
                    nk = 4 * s + 4
                    qT = qTa if s < 2 else qTb
                    qoff = (s % 2) * QW
                    for kb in range(nk):
                        # Diagonal tiles: queries f < 64*t_idx are fully
                        # masked; trim scores/ctx to the live range. The
                        # full-width mask add + exp turn the trimmed (stale
                        # but finite) columns into exact 0s.
                        t_idx = kb - (nk - 4)
                        off = 64 * t_idx if t_idx > 0 else 0
                        ps_sc = rot.tile([128, 512], F32, name="rps", tag="rps")
                        # contraction order kxb-chunks first (kxb lands
                        # before kxa in the DMA schedule)
                        for ci, ec in enumerate(list(range(4, 8)) + list(range(4))):
                            src = kxa if ec < 4 else kxb
                            mm = nc.tensor.matmul(
                                ps_sc[:, off:QW],
                                src[:, (ec % 4) * S + kb * 128:
                                    (ec % 4) * S + (kb + 1) * 128],
                                qT[ec][:, qoff + off:qoff + QW],
                                start=(ci == 0), stop=(ci == NE - 1),
                            )
                            if kb == 0 and ci == 0 and s < 3:
                                # xkd group s+2 launches as slot s starts
                                # (groups 0-1 = slot 0, dispatched upfront)
                                delayed_dma(xkdg[s + 2][:, :],
                                            xkdg_d[s + 2][:, :],
                                            mm, f"xkd{s + 2} at slot{s}")
                        if t_idx >= 0:
                            nc.vector.tensor_add(
                                ps_sc[:, 0:QW], ps_sc[:, 0:QW],
                                mask_t[:, t_idx * QW:(t_idx + 1) * QW],
                            )
                        pt = ptp.tile([128, QW], BF16, name="pt", tag="pt")
                        nc.scalar.activation(pt[:, :], ps_sc[:, 0:QW], EXP,
                                             scale=SCALE)
                        pending.append((s, kb, off, pt))
                        if len(pending) > DEPTH:
                            consume(pending.popleft())
                while pending:
                    consume(pending.popleft())

    nc.compile()
    return nc


def _get_nc():
    if "nc" not in _cached:
        _cached["nc"] = _build()
    return _cached["nc"]


def build_in_maps(x, W_q, W_k, W_v):
    import ml_dtypes

    BF = ml_dtypes.bfloat16
    x = np.asarray(x, dtype=np.float32)
    wq = np.asarray(W_q, dtype=np.float32)
    wk = np.asarray(W_k, dtype=np.float32)
    m = np.ascontiguousarray((wq @ wk.T).astype(BF))
    wv_f = np.asarray(W_v, dtype=np.float32).astype(BF)
    # wv host layout: [128, 8*1024], column block dc = Wv[dc*128:(dc+1)*128, :]
    wv = np.ascontiguousarray(
        np.concatenate([wv_f[dc * 128:(dc + 1) * 128, :] for dc in range(NE)],
                       axis=1))
    ones = np.ones((128, 2), dtype=BF)

    p = np.arange(128, dtype=np.int64)[:, None]
    f = np.arange(QW, dtype=np.int64)[None, :]
    masks_h = []
    for h in range(2):
        tiles = [
            np.where(128 * t + p <= 2 * f + h, np.float32(0.0), np.float32(MASK_NEG))
            for t in range(4)
        ]
        masks_h.append(np.concatenate(tiles, axis=1).astype(np.float32))

    xbf = x.astype(BF)
    kxa_b, kxb_b, xkdg_b = [], [], []
    for b in range(B):
        xtr = np.ascontiguousarray(xbf[b].T)         # [D, S]
        kxa_b.append(np.ascontiguousarray(np.concatenate(
            [xtr[ec * 128:(ec + 1) * 128, :] for ec in range(4)], axis=1)))
        kxb_b.append(np.ascontiguousarray(np.concatenate(
            [xtr[ec * 128:(ec + 1) * 128, :] for ec in range(4, 8)], axis=1)))
        xk = xbf[b].reshape(NKBLK, 128, D)
        xkdg_b.append([np.ascontiguousarray(
            np.concatenate([xk[i] for i in range(lo, hi)], axis=1))
            for lo, hi in [(0, 2), (2, 4), (4, 8), (8, 12), (12, 16)]])

    in_maps = []
    for c in range(8):
        b, h = c // 2, c % 2
        xq = xbf[b, h::2, :].T                        # [D, OWNQ]
        # mqx rows dc*128..: [m_dc | xq_dc], each [128, 1024+1024]
        mqx = np.ascontiguousarray(np.concatenate([
            np.concatenate([m[dc * 128:(dc + 1) * 128, :],
                            xq[dc * 128:(dc + 1) * 128, :]], axis=1)
            for dc in range(NE)], axis=0))
        im = {
            "mqx": mqx,
            "kxa": kxa_b[b],
            "kxb": kxb_b[b],
            "wv": wv,
            "masks": masks_h[h],
            "ones": ones,
        }
        for g in range(5):
            im[f"xkd{g}"] = xkdg_b[b][g]
        in_maps.append(im)
    return in_maps


def kernel(x, W_q, W_k, W_v):
    from concourse.bass_utils import run_bass_kernel_spmd

    in_maps = build_in_maps(x, W_q, W_k, W_v)
    nc = _get_nc()
    res = run_bass_kernel_spmd(nc, in_maps, core_ids=list(range(8)))

    out = np.empty((B, S, D), dtype=np.float32)
    for c in range(8):
        b, h = c // 2, c % 2
        out[b, h::2, :] = res.results[c]["o"]
    return out
